# revision 8
# baseline (speedup 1.0000x reference)
"""Trainium2 Bass kernel: dense soft-MoE (router MLP + 8 expert MLPs + gated combine).

All matmuls in bf16 (rel err ~4.4e-3 vs the 2e-2 budget; PSUM accumulates
fp32). Evolution from the fp32r baseline (1010us -> 816us):
  - bf16 operands everywhere: same PE rate (1 cycle/row) as fp32r but FWL
    halves LDWEIGHTS, weight DMA halves, SBUF pressure halves.
  - weights host-prepacked into exact SBUF layout -> one contiguous DMA per
    weight tensor per expert; x is SBUF-resident for the whole kernel.
  - no bias matmuls: bout/rbout host-tiled across partitions, applied on DVE.
  - PE warmup matmuls lift the HAM clock gate during startup DMAs.
  - router fused into expert-0's chunk loop; DMA emission ordered by first
    use; y streamed out per token tile at the last expert.
Details:
  - router fused into expert 0's chunk loop: router chunk ch runs right
    before expert 0 processes chunk ch, so the PE chews on expert-0 matmuls
    while the later x chunks / router weights are still streaming in (v3
    stalled ~8us at startup waiting on x DMAs between router chunks).
  - DMA emission order matches first use: r1,x0,w1_e0 first.
  - PSUM pools restructured: psL=5 banks also serves the router logit
    groups; pso slots bank-aligned (512-wide) to avoid half-bank collisions.
"""

import sys

if "/opt/trn_rl_repo" not in sys.path:
    sys.path.insert(0, "/opt/trn_rl_repo")

import ml_dtypes
import numpy as np

import concourse.mybir as mybir
import concourse.tile as tile
from concourse import bacc, bass_utils

N_CORES = 8
N_TOKENS = 16384
T = N_TOKENS // N_CORES  # 2048 tokens per core
D, W, O, E, R = 512, 1024, 256, 8, 256
NC = 512  # token chunk through one layer stack
P = 128
N_CHUNKS = T // NC  # 4
TT_PER_CHUNK = NC // P  # 4
N_TT = T // P  # 16 token tiles per core

F32 = mybir.dt.float32
BF16 = mybir.dt.bfloat16
NPBF16 = ml_dtypes.bfloat16
AF = mybir.ActivationFunctionType
ALU = mybir.AluOpType
AX = mybir.AxisListType


def _build():
    nc = bacc.Bacc("TRN2", target_bir_lowering=False)

    # all prepacked host-side into SBUF layout [128, ko, free]
    xsb_d = nc.dram_tensor("xsb", [N_CHUNKS, P, D // P, NC], BF16, kind="ExternalInput")
    w1_d = nc.dram_tensor("w1p", [E, P, D // P, W], BF16, kind="ExternalInput")
    w2_d = nc.dram_tensor("w2p", [E, P, W // P, W], BF16, kind="ExternalInput")
    wo_d = nc.dram_tensor("wop", [E, P, W // P, O], BF16, kind="ExternalInput")
    b1_d = nc.dram_tensor("b1p", [E, P, W // P], F32, kind="ExternalInput")
    b2_d = nc.dram_tensor("b2p", [E, P, W // P], F32, kind="ExternalInput")
    bo_d = nc.dram_tensor("bop", [P, E, O], F32, kind="ExternalInput")  # host-tiled
    r1_d = nc.dram_tensor("r1p", [P, D // P, R], BF16, kind="ExternalInput")
    r2_d = nc.dram_tensor("r2p", [P, R // P, R], BF16, kind="ExternalInput")
    ro_d = nc.dram_tensor("rop", [P, R // P, E], BF16, kind="ExternalInput")
    rb1_d = nc.dram_tensor("rb1p", [P, R // P], F32, kind="ExternalInput")
    rb2_d = nc.dram_tensor("rb2p", [P, R // P], F32, kind="ExternalInput")
    rbo_d = nc.dram_tensor("rbop", [P, E], F32, kind="ExternalInput")  # host-tiled
    y = nc.dram_tensor("y", [T, O], F32, kind="ExternalOutput")

    with tile.TileContext(nc) as tc:
        with (
            tc.tile_pool(name="persist", bufs=1) as persist,
            tc.tile_pool(name="smallp", bufs=2) as smallp,
            tc.tile_pool(name="ap", bufs=2) as ap,  # a1 / h1 / h2 share slots
            tc.tile_pool(name="a2p", bufs=2) as a2p,
            tc.tile_pool(name="wp", bufs=2) as wp,
            tc.tile_pool(name="psL", bufs=4, space="PSUM") as psL,
            tc.tile_pool(name="psS", bufs=2, space="PSUM") as psS,
            tc.tile_pool(name="psG", bufs=2, space="PSUM") as psG,
        ):
            # ---- PE warmup: DMA-independent dummy matmuls to lift the HAM
            # clock gate (4/8 -> 8/8) while the startup DMAs stream in ----
            warm = persist.tile([P, NC], BF16, name="warm")
            nc.vector.memset(warm[:], 0.0)
            wps = psL.tile([P, NC], F32, name="ps")
            for i in range(14):
                nc.tensor.matmul(
                    wps[:, : 2 * P],
                    warm[:, :P],
                    warm[:, : 2 * P],
                    start=(i == 0),
                    stop=(i == 13),
                )

            # ---- one-time loads: the two first-MM-critical transfers (r1, x0)
            # get the first DMA issue slots; tiny bias rows wait ----
            r1sb = persist.tile([P, D // P, R], BF16, name="r1sb")
            nc.sync.dma_start(r1sb[:], r1_d[:])
            xsb = []
            xt = persist.tile([P, D // P, NC], BF16, name="xsb0")
            nc.sync.dma_start(xt[:], xsb_d[0])
            xsb.append(xt)
            rb1sb = persist.tile([P, R // P], F32, name="rb1sb")
            nc.sync.dma_start(rb1sb[:], rb1_d[:])
            # expert-0 weights, interleaved in order of first PE use
            e0 = {}
            e0["w1t"] = wp.tile([P, D // P, W], BF16, name="w1t")
            nc.sync.dma_start(e0["w1t"][:], w1_d[0])
            r2sb = persist.tile([P, R // P, R], BF16, name="r2sb")
            nc.sync.dma_start(r2sb[:], r2_d[:])
            rb2sb = persist.tile([P, R // P], F32, name="rb2sb")
            nc.sync.dma_start(rb2sb[:], rb2_d[:])
            e0["b1t"] = wp.tile([P, W // P], F32, name="b1t")
            nc.sync.dma_start(e0["b1t"][:], b1_d[0])
            routsb = persist.tile([P, R // P, E], BF16, name="routsb")
            nc.sync.dma_start(routsb[:], ro_d[:])
            rbosb = persist.tile([P, E], F32, name="rbosb")
            nc.sync.dma_start(rbosb[:], rbo_d[:])
            e0["w2t"] = wp.tile([P, W // P, W], BF16, name="w2t")
            nc.sync.dma_start(e0["w2t"][:], w2_d[0])
            e0["b2t"] = wp.tile([P, W // P], F32, name="b2t")
            nc.sync.dma_start(e0["b2t"][:], b2_d[0])
            for ch in range(1, N_CHUNKS):
                xt = persist.tile([P, D // P, NC], BF16, name=f"xsb{ch}")
                nc.sync.dma_start(xt[:], xsb_d[ch])
                xsb.append(xt)
                if ch == 1:
                    e0["wot"] = wp.tile([P, W // P, O], BF16, name="wot")
                    nc.sync.dma_start(e0["wot"][:], wo_d[0])
                    bosb = persist.tile([P, E, O], F32, name="bosb")
                    nc.sync.dma_start(bosb[:], bo_d[:])

            gates = persist.tile([P, N_TT, E], F32, name="gates")
            acc = persist.tile([P, N_TT, O], F32, name="acc")

            def router_chunk(ch):
                h1 = ap.tile([P, W // P, NC], BF16, name="act")[:, : R // P, :]
                for fo in range(R // P):
                    ps = psL.tile([P, NC], F32, name="ps")
                    for ko in range(D // P):
                        nc.tensor.matmul(
                            ps[:],
                            r1sb[:, ko, fo * P : (fo + 1) * P],
                            xsb[ch][:, ko, :],
                            start=(ko == 0),
                            stop=(ko == D // P - 1),
                        )
                    nc.scalar.activation(
                        h1[:, fo, :], ps[:], AF.Relu, bias=rb1sb[:, fo : fo + 1]
                    )
                h2 = ap.tile([P, W // P, NC], BF16, name="act")[:, : R // P, :]
                for fo in range(R // P):
                    ps = psL.tile([P, NC], F32, name="ps")
                    for ko in range(R // P):
                        nc.tensor.matmul(
                            ps[:],
                            r2sb[:, ko, fo * P : (fo + 1) * P],
                            h1[:, ko, :],
                            start=(ko == 0),
                            stop=(ko == R // P - 1),
                        )
                    nc.scalar.activation(
                        h2[:, fo, :], ps[:], AF.Relu, bias=rb2sb[:, fo : fo + 1]
                    )
                return h2

            def router_logits(ch, h2, tt):
                # one token tile of logits + softmax (sparse PE work -
                # interleaved into expert-0's dense L1 stream so the HAM
                # activity monitor never sees an idle window and re-throttles)
                gt = ch * TT_PER_CHUNK + tt
                tsl = slice(tt * P, (tt + 1) * P)
                ps8 = psG.tile([P, E], F32, name="ps8")
                for ko in range(R // P):
                    nc.tensor.matmul(
                        ps8[:],
                        h2[:, ko, tsl],
                        routsb[:, ko, :],
                        start=(ko == 0),
                        stop=(ko == R // P - 1),
                    )
                nc.vector.tensor_tensor(
                    ps8[:], ps8[:], rbosb[:], ALU.add
                )
                st = smallp.tile([P, 16], F32, name="st")
                mx, sm, rs, eg = st[:, 0:1], st[:, 1:2], st[:, 2:3], st[:, 8:16]
                nc.vector.reduce_max(mx, ps8[:], axis=AX.X, negate=True)
                nc.scalar.activation(eg, ps8[:], AF.Exp, bias=mx)
                nc.vector.reduce_sum(sm, eg, axis=AX.X)
                nc.vector.reciprocal(rs, sm)
                nc.vector.tensor_scalar_mul(gates[:, gt, :], eg, rs)

            # ---------------- Experts ----------------
            def l3_group(le, lch, la2, lwot, tt):
                gt = lch * TT_PER_CHUNK + tt
                tsl = slice(tt * P, (tt + 1) * P)
                pso = psS.tile([P, O], F32, name="pso")
                for ko in range(W // P):
                    nc.tensor.matmul(
                        pso[:],
                        la2[:, ko, tsl],
                        lwot[:, ko, :],
                        start=(ko == 0),
                        stop=(ko == W // P - 1),
                    )
                g = gates[:, gt, le : le + 1]
                if le == 0:
                    nc.vector.tensor_scalar_mul(acc[:, gt, :], pso[:], g)
                else:
                    nc.vector.scalar_tensor_tensor(
                        acc[:, gt, :], pso[:], g, acc[:, gt, :], ALU.mult, ALU.add
                    )
                nc.vector.scalar_tensor_tensor(
                    acc[:, gt, :], bosb[:, le, :], g, acc[:, gt, :], ALU.mult, ALU.add
                )
                if le == E - 1:
                    nc.sync.dma_start(
                        y[:].rearrange("(gt p) o -> p gt o", p=P)[:, gt, :],
                        acc[:, gt, :],
                    )

            pend = None
            for e in range(E):
                if e == 0:
                    w1t, b1t, w2t, b2t, wot = (
                        e0["w1t"], e0["b1t"], e0["w2t"], e0["b2t"], e0["wot"]
                    )
                else:
                    w1t = wp.tile([P, D // P, W], BF16, name="w1t")
                    nc.sync.dma_start(w1t[:], w1_d[e])
                    b1t = wp.tile([P, W // P], F32, name="b1t")
                    nc.sync.dma_start(b1t[:], b1_d[e])
                    w2t = wp.tile([P, W // P, W], BF16, name="w2t")
                    nc.sync.dma_start(w2t[:], w2_d[e])
                    b2t = wp.tile([P, W // P], F32, name="b2t")
                    nc.sync.dma_start(b2t[:], b2_d[e])
                    wot = wp.tile([P, W // P, O], BF16, name="wot")
                    nc.sync.dma_start(wot[:], wo_d[e])

                for ch in range(N_CHUNKS):
                    h2 = router_chunk(ch) if e == 0 else None
                    a1 = ap.tile([P, W // P, NC], BF16, name="act")
                    for fo in range(W // P):
                        ps = psL.tile([P, NC], F32, name="ps")
                        for ko in range(D // P):
                            nc.tensor.matmul(
                                ps[:],
                                w1t[:, ko, fo * P : (fo + 1) * P],
                                xsb[ch][:, ko, :],
                                start=(ko == 0),
                                stop=(ko == D // P - 1),
                            )
                        nc.scalar.activation(
                            a1[:, fo, :], ps[:], AF.Relu, bias=b1t[:, fo : fo + 1]
                        )
                        # interleave sparse work into the dense L1 stream:
                        # even fo: this chunk's router logits (expert 0 only);
                        # odd fo: deferred previous-chunk L3 group
                        if h2 is not None and fo % 2 == 0:
                            router_logits(ch, h2, fo // 2)
                        if pend is not None and fo % 2 == 1:
                            l3_group(*pend, fo // 2)
                    if pend is not None:
                        pend = None
                    a2 = a2p.tile([P, W // P, NC], BF16, name="a2")
                    for fo in range(W // P):
                        ps = psL.tile([P, NC], F32, name="ps")
                        for ko in range(W // P):
                            nc.tensor.matmul(
                                ps[:],
                                w2t[:, ko, fo * P : (fo + 1) * P],
                                a1[:, ko, :],
                                start=(ko == 0),
                                stop=(ko == W // P - 1),
                            )
                        nc.scalar.activation(
                            a2[:, fo, :], ps[:], AF.Relu, bias=b2t[:, fo : fo + 1]
                        )
                    pend = (e, ch, a2, wot)

            for tt in range(TT_PER_CHUNK):
                l3_group(*pend, tt)

    nc.compile()
    return nc


_CACHED_NC = None


def _get_nc():
    global _CACHED_NC
    if _CACHED_NC is None:
        _CACHED_NC = _build()
    return _CACHED_NC


def _pack_k(a, ko):
    """[K, F] -> [128, K//128, F] with k = ko*128 + p."""
    return np.ascontiguousarray(a.reshape(ko, P, -1).transpose(1, 0, 2))


def _pack_bias(b, fo):
    """[F] -> [128, F//128] with f = fo*128 + p."""
    return np.ascontiguousarray(b.reshape(fo, P).T)


def make_in_maps(inputs):
    f32 = {k: np.asarray(v, dtype=np.float32) for k, v in inputs.items()}
    bf = lambda a: np.ascontiguousarray(a.astype(NPBF16))

    shared = {
        "w1p": bf(np.stack([_pack_k(f32["w1"][e], D // P) for e in range(E)])),
        "w2p": bf(np.stack([_pack_k(f32["w2"][e], W // P) for e in range(E)])),
        "wop": bf(np.stack([_pack_k(f32["wout"][e], W // P) for e in range(E)])),
        "b1p": np.ascontiguousarray(
            np.stack([_pack_bias(f32["b1"][e], W // P) for e in range(E)])
        ),
        "b2p": np.ascontiguousarray(
            np.stack([_pack_bias(f32["b2"][e], W // P) for e in range(E)])
        ),
        "bop": np.ascontiguousarray(np.broadcast_to(f32["bout"][None], (P, E, O))),
        "r1p": bf(_pack_k(f32["r1"], D // P)),
        "r2p": bf(_pack_k(f32["r2"], R // P)),
        "rop": bf(_pack_k(f32["rout"], R // P)),
        "rb1p": np.ascontiguousarray(_pack_bias(f32["rb1"], R // P)),
        "rb2p": np.ascontiguousarray(_pack_bias(f32["rb2"], R // P)),
        "rbop": np.ascontiguousarray(np.broadcast_to(f32["rbout"][None], (P, E))),
    }
    x = f32["x"]
    in_maps = []
    for c in range(N_CORES):
        xs = x[c * T : (c + 1) * T]  # [T, D]
        xp = np.stack(
            [
                _pack_k(np.ascontiguousarray(xs[ch * NC : (ch + 1) * NC].T), D // P)
                for ch in range(N_CHUNKS)
            ]
        )
        m = {"xsb": bf(xp)}
        m.update(shared)
        in_maps.append(m)
    return in_maps


def kernel(**inputs):
    in_maps = make_in_maps(inputs)
    nc = _get_nc()
    res = bass_utils.run_bass_kernel_spmd(nc, in_maps, core_ids=list(range(N_CORES)))
    return np.concatenate([res.results[c]["y"] for c in range(N_CORES)], axis=0)
